# revision 1
# baseline (speedup 1.0000x reference)
"""Euclidean distance layer (retrieval kNN) on 8 Trainium2 NeuronCores.

out[b, o] = || x[b, :] - weight[:, o] ||_2   for x [2048, 1024], weight [1024, 16384].

Strategy (sharding_hint): shard output columns across the 8 cores (2048 each).
Per core, compute d2 = x2[b] + w2[o] - 2 * (x @ w_shard) and out = sqrt(d2):
  - the big matmul runs in fp8e4 with DoubleRow perf mode (2 MACs/cell/cycle,
    8x the fp32 rate; its rounding is attenuated ~64x in the output because
    |2xw| << d2); each instruction contracts a pair of K=128 tiles
  - every PSUM accumulation group is seeded with -w2/2 broadcast to all
    partitions by a DoubleRow ones-matmul against a [(-w2/2); 0] fp8 row pair,
    so the epilogue needs no elementwise add
  - w2 = colsum(w^2) itself comes from a (-1/2)-constant stationary matmul
    over bf16 squares (reduction + partition-broadcast in one PE op)
  - x2 = rowsum(x^2) is one DVE tensor_tensor_reduce per row tile on an fp16
    copy of x
  - epilogue per [128, 512] tile is a single ACT sqrt(-2*psum + x2_bias)
Host side only transposes/shards/casts inputs and reassembles the output.
"""
import numpy as np

import concourse.bass as bass
import concourse.tile as tile
from concourse import bacc, mybir
from concourse.bass_utils import run_bass_kernel_spmd

f32 = mybir.dt.float32
f32r = mybir.dt.float32r
f16 = mybir.dt.float16
bf16 = mybir.dt.bfloat16
AF = mybir.ActivationFunctionType

B = 2048      # batch rows
I = 1024      # input size (contraction)
O = 16384     # output size (prototype count)
N_CORES = 8
OS = O // N_CORES   # 2048 output columns per core
P = 128       # partitions
NB = 512      # moving free-dim per matmul / psum bank
KT = I // P   # 8 k-tiles
MT = B // P   # 16 m-tiles
NT = OS // NB  # 4 n-blocks

fp8 = mybir.dt.float8e4
MM_DT = fp8           # matmul input dtype: fp8 (DoubleRow), bf16, or f32r
DR = mybir.MatmulPerfMode.DoubleRow if MM_DT is fp8 else None


def _emit_body(nc, tc, x_d, xt_d, w_d, out_d):
    from contextlib import ExitStack
    with ExitStack() as ctx:
        const_p = ctx.enter_context(tc.tile_pool(name="const", bufs=1))
        xt_p = ctx.enter_context(tc.tile_pool(name="xt", bufs=1))
        w_p = ctx.enter_context(tc.tile_pool(name="w", bufs=1))
        xr_p = ctx.enter_context(tc.tile_pool(name="xr", bufs=1))
        sq_p = ctx.enter_context(tc.tile_pool(name="sq", bufs=2))
        wsq_p = ctx.enter_context(tc.tile_pool(name="wsq", bufs=4))
        w2_p = ctx.enter_context(tc.tile_pool(name="w2", bufs=1))
        x2_p = ctx.enter_context(tc.tile_pool(name="x2", bufs=1))
        o_p = ctx.enter_context(tc.tile_pool(name="o", bufs=6))
        o32_p = ctx.enter_context(tc.tile_pool(name="o32", bufs=4))
        ps_p = ctx.enter_context(tc.tile_pool(name="ps", bufs=6, space="PSUM"))
        psw2_p = ctx.enter_context(tc.tile_pool(name="psw2", bufs=2, space="PSUM"))

        neghalf = const_p.tile([P, P], bf16)
        nc.vector.memset(neghalf[:], -0.5)
        ones8 = const_p.tile([1, 2, P], fp8)    # DoubleRow preload stationary
        nc.vector.memset(ones8[:], 1.0)

        xt_sb = xt_p.tile([P, KT, B], MM_DT)    # x.T resident, matmul stationary
        w_sb = w_p.tile([P, KT, OS], MM_DT)     # w shard resident, matmul moving
        xr_sb = xr_p.tile([P, MT, I], f16)      # x rows (fp16) for x2
        w2pair = w2_p.tile([1, 2, OS], fp8)     # [-w2/2; zeros] preload rhs rows
        x2col = x2_p.tile([P, MT], f32)         # x2 per-partition, one col per m-tile

        xt_src = xt_d.ap().rearrange("(k p) b -> p k b", p=P)    # [128, KT, B]
        w_src = w_d.ap().rearrange("(k p) o -> p k o", p=P)      # [128, KT, OS]
        x_src = x_d.ap().rearrange("(m p) i -> p m i", p=P)      # [128, MT, I]

        def dma_w_chunk(n, split=1):
            ns = slice(n * NB, (n + 1) * NB)
            kstep = KT // split
            for k0 in range(0, KT, kstep):
                nc.sync.dma_start(w_sb[:, k0:k0 + kstep, ns],
                                  w_src[:, k0:k0 + kstep, ns])

        def dma_xt_chunk(c):
            nc.sync.dma_start(xt_sb[:, :, c * NB:(c + 1) * NB],
                              xt_src[:, :, c * NB:(c + 1) * NB])

        def dma_x_rows(m0, m1):
            nc.sync.dma_start(xr_sb[:, m0:m1, :], x_src[:, m0:m1, :])

        # input DMAs, ordered so the PE's earliest dependencies land first:
        # the main loop runs (n-block, m-half) super-blocks, so block 0 only
        # needs w chunk 0 + half of xt + half of x.
        dma_w_chunk(0, split=4)
        dma_xt_chunk(0)
        dma_x_rows(0, 4)
        dma_xt_chunk(1)
        dma_w_chunk(1)
        dma_x_rows(4, 8)
        dma_xt_chunk(2)
        dma_xt_chunk(3)
        dma_w_chunk(2)
        dma_x_rows(8, 16)
        dma_w_chunk(3)

        sq_dt = f32 if MM_DT is f32r else MM_DT
        nc.vector.memset(w2pair[:], 0.0)

        def emit_w2(n):
            # psw2 = -0.5 * colsum(w^2) broadcast across partitions
            ns = slice(n * NB, (n + 1) * NB)
            psw2 = psw2_p.tile([P, NB], f32)
            for k in range(KT):
                wsq = wsq_p.tile([P, NB], bf16)
                nc.vector.tensor_mul(wsq[:], w_sb[:, k, ns].bitcast(sq_dt),
                                     w_sb[:, k, ns].bitcast(sq_dt))
                nc.tensor.matmul(psw2[:], neghalf[:], wsq[:],
                                 start=(k == 0), stop=(k == KT - 1))
            nc.vector.tensor_copy(w2pair[:, 0, ns], psw2[0:1, :])

        blocks = [(n, h) for n in range(NT) for h in range(2)]
        # w2(n) must precede block 2n (first use) but trail its w-chunk DMA:
        w2_at = {0: 0, 1: 1, 3: 2, 5: 3}
        for bi, (n, h) in enumerate(blocks):
            if bi in w2_at:
                emit_w2(w2_at[bi])
            ns = slice(n * NB, (n + 1) * NB)
            osb = None
            for m in range(h * (MT // 2), (h + 1) * (MT // 2)):
                if n == 0:
                    sq = sq_p.tile([P, I], f32)
                    nc.vector.scalar_tensor_tensor(
                        sq[:], xr_sb[:, m, :], 1.0, xr_sb[:, m, :],
                        op0=mybir.AluOpType.mult, op1=mybir.AluOpType.mult,
                        accum_out=x2col[:, m:m + 1])
                if m % 2 == 0:
                    osb = o_p.tile([P, 2, NB], f16)
                ps = ps_p.tile([P, NB], f32)
                # seed the group with -w2/2 broadcast via a DoubleRow
                # ones-matmul (same perf mode as the data matmuls)
                nc.tensor.matmul(ps[:], ones8[:], w2pair[:, :, ns],
                                 start=True, stop=False, perf_mode=DR,
                                 skip_group_check=True)
                for j in range(KT // 2):
                    nc.tensor.matmul(ps[:],
                                     xt_sb[:, 2 * j:2 * j + 2, m * P:(m + 1) * P],
                                     w_sb[:, 2 * j:2 * j + 2, ns],
                                     start=False, stop=(j == KT // 2 - 1),
                                     perf_mode=DR, skip_group_check=True)
                o32 = o32_p.tile([P, NB], f32)
                nc.scalar.activation(o32[:], ps[:], AF.Sqrt,
                                     bias=x2col[:, m:m + 1], scale=-2.0)
                # encode as fp16 around the distance mean: |out-32| ~ 0.7, so
                # fp16 error lands at ~5e-4 relative to the deviation (the
                # direct-fp16 path at magnitude 32 would be 30x coarser);
                # alternate engines so neither becomes the bottleneck
                eng = nc.vector if (n * MT + m) % 4 == 3 else nc.gpsimd
                eng.tensor_scalar_sub(osb[:, m % 2, :], o32[:], 32.0)
                if m % 2 == 1:
                    g = m // 2
                    dst = out_d.ap()[n, g * 2 * P:(g + 1) * 2 * P, :].rearrange(
                        "(mm p) j -> p mm j", p=P)
                    nc.sync.dma_start(dst, osb[:])


def build(repeats=1):
    nc = bacc.Bacc("TRN2", target_bir_lowering=False, debug=False,
                   num_devices=N_CORES)
    x_d = nc.dram_tensor("x", [B, I], f16, kind="ExternalInput")
    xt_d = nc.dram_tensor("xt", [I, B], MM_DT, kind="ExternalInput")
    w_d = nc.dram_tensor("w", [I, OS], MM_DT, kind="ExternalInput")
    out_d = nc.dram_tensor("out", [NT, B, NB], f16, kind="ExternalOutput")
    with tile.TileContext(nc) as tc:
        for _ in range(repeats):
            _emit_body(nc, tc, x_d, xt_d, w_d, out_d)
    nc.compile()
    return nc


_NC = None


def _mm_np(a):
    """Cast a float32 array to the matmul host dtype."""
    import ml_dtypes
    if MM_DT is f32r:
        return np.ascontiguousarray(a, dtype=np.float32)
    if MM_DT is fp8:
        return np.ascontiguousarray(np.asarray(a).astype(ml_dtypes.float8_e4m3))
    return np.ascontiguousarray(np.asarray(a).astype(ml_dtypes.bfloat16))


def make_in_maps(x, weight):
    x16 = np.ascontiguousarray(x.astype(np.float16))
    xt = _mm_np(x.T)
    return [{"x": x16, "xt": xt,
             "w": _mm_np(weight[:, c * OS:(c + 1) * OS])}
            for c in range(N_CORES)]


def assemble(results):
    cols = []
    for c in range(N_CORES):
        blk = results[c]["out"].astype(np.float32) + 32.0   # undo fp16 shift-encode
        cols.append(blk.transpose(1, 0, 2).reshape(B, OS))
    return np.ascontiguousarray(np.concatenate(cols, axis=1))


def kernel(x, weight):
    global _NC
    x = np.asarray(x, dtype=np.float32)
    weight = np.asarray(weight, dtype=np.float32)
    if _NC is None:
        _NC = build(repeats=1)
    in_maps = make_in_maps(x, weight)
    res = run_bass_kernel_spmd(_NC, in_maps, core_ids=list(range(N_CORES)))
    return assemble(res.results)



# revision 41
# speedup vs baseline: 4.6979x; 4.6979x over previous
"""Euclidean distance layer (retrieval kNN) on 8 Trainium2 NeuronCores.

out[b, o] = || x[b, :] - weight[:, o] ||_2   for x [2048, 1024], weight [1024, 16384].

Strategy (sharding_hint): shard output columns across the 8 cores (2048 each).
Per core, compute d2 = x2[b] + w2[o] - 2 * (x @ w_shard) and out = sqrt(d2):
  - the big matmul runs in fp8e4 with DoubleRow perf mode; each instruction
    contracts a pair of K=128 tiles (its rounding is attenuated ~64x in the
    output because |2xw| << d2); j-outer/n-inner order loads each stationary
    once per (m, k-pair)
  - the -w2/2 seed is FOLDED INTO the last DR pair: the device sets x.T row
    896 (k-tile 7, partition 0) to ones and writes -w2/2 into w row 896
    after computing w2, so every accumulation group gets x2+w2-2xw with no
    extra seed matmul; the dropped x[:,896]*w[896,:] term is ~1e-4 rel
  - w2 = colsum(w^2): DVE squares k 0..3 as fp8 32*w^2 (scalar_tensor_tensor
    is DVE-only — walrus rejects it on Pool) reduced with DoubleRow against
    a -1 stationary; Pool squares k 4..7 as plain bf16 w^2 reduced against a
    -32 bf16 stationary into the same psum group
  - x2 = rowsum(x^2) from fp8 squares of x.T scaled by 1.75 (de-phases the
    fp8 square-of-grid rounding bias; divided back out in the x2col copy),
    used as matmul STATIONARY against a [P,2,1] DR ones moving so each
    k-sum lands straight in a psum column per m-tile
  - epilogue per m-tile: two ACT sqrt(-2*psum + x2_bias) over 2 psum banks
    each ([128,1024]), writing fp16 directly; out DMA every 4 m-tiles
  - no fp16 copy of x is shipped at all: 12MB HBM/body (xt 2 + w 2 + out 8);
    DRAM layouts are host-pre-rearranged so every DMA moves 4-16KB
    contiguous runs per partition (descriptor-efficient)
  - PSUM: main pool 3x2 banks, prologue pool 1x2 banks. The prologue pool
    frees early in each body, so body i+1's whole w2/x2/seed prologue is
    emitted INTERLEAVED into body i's main loop and runs during it —
    without this the ACT stream starves ~10us at every body boundary
  - input/output/x2col pools are triple-buffered so body i+2's DMAs carry
    no tile WAR wait and fill every idle slot of the DMA device
Host side only transposes/shards/casts inputs and reassembles the output.
"""
import numpy as np

import concourse.bass as bass
import concourse.tile as tile
from concourse import bacc, mybir
from concourse.bass_utils import run_bass_kernel_spmd

f32 = mybir.dt.float32
f16 = mybir.dt.float16
bf16 = mybir.dt.bfloat16
fp8 = mybir.dt.float8e4
AF = mybir.ActivationFunctionType
MUL = mybir.AluOpType.mult

B = 2048      # batch rows
I = 1024      # input size (contraction)
O = 16384     # output size (prototype count)
N_CORES = 8
OS = O // N_CORES   # 2048 output columns per core
P = 128       # partitions
NB = 512      # psum bank width in f32
KT = I // P   # 8 k-tiles
MT = B // P   # 16 m-tiles
NT = OS // NB  # 4 n-blocks
JT = KT // 2  # 4 DoubleRow k-pairs

DR = mybir.MatmulPerfMode.DoubleRow


def _make_pools(nc, tc, ctx):
    pools = dict(
        const_p=ctx.enter_context(tc.tile_pool(name="const", bufs=1)),
        xt_p=ctx.enter_context(tc.tile_pool(name="xt", bufs=3)),
        w_p=ctx.enter_context(tc.tile_pool(name="w", bufs=3)),
        wsq_p=ctx.enter_context(tc.tile_pool(name="wsq", bufs=5)),
        xsq_p=ctx.enter_context(tc.tile_pool(name="xsq", bufs=5)),
        x2_p=ctx.enter_context(tc.tile_pool(name="x2", bufs=3)),
        o_p=ctx.enter_context(tc.tile_pool(name="o", bufs=3)),
        ps_p=ctx.enter_context(tc.tile_pool(name="ps", bufs=3, space="PSUM")),
        pro_p=ctx.enter_context(tc.tile_pool(name="pro", bufs=1, space="PSUM")),
    )
    negone = pools["const_p"].tile([P, 2, P], fp8)  # w2 sum, DR over 32*w^2
    nc.vector.memset(negone[:], -1.0)
    neg32 = pools["const_p"].tile([P, P], bf16)     # w2 sum, bf16 over w^2
    nc.vector.memset(neg32[:], -32.0)
    ones_mv = pools["const_p"].tile([P, 2, 1], fp8)  # DR moving for x2 sum
    nc.vector.memset(ones_mv[:], 1.0)
    pools["negone"] = negone
    pools["neg32"] = neg32
    pools["ones_mv"] = ones_mv
    return pools


def _emit_inputs(nc, tc, pp, xt_d, w_d):
    """Allocate this body's input tiles and issue their DMAs at high
    scheduler priority so the next body's inputs transfer during the
    current body's DMA-idle window."""
    xt_sb = pp["xt_p"].tile([P, KT, B], fp8)    # x.T resident
    w_sb = pp["w_p"].tile([P, KT, OS], fp8)     # w shard resident

    # DRAM layouts are host-pre-rearranged so every DMA reads 4-8KB
    # contiguous runs per partition (descriptor-efficient):
    #   xt_d [2, P, KT, B/2] (b-half major), w_d [NT, P, KT, NB]
    with tc.high_priority(offset=800):
        nc.sync.dma_start(xt_sb[:, :, 0:B // 2], xt_d.ap()[0])
        for n in range(NT):
            ns = slice(n * NB, (n + 1) * NB)
            nc.sync.dma_start(w_sb[:, :, ns], w_d.ap()[n])
            if n == 0:
                nc.sync.dma_start(xt_sb[:, :, B // 2:B], xt_d.ap()[1])
    return xt_sb, w_sb


def _prologue_chunks(nc, pp, handles):
    """Build the w2/seed/x2 prologue for one body as a list of emission
    closures, so the caller can interleave them into the previous body's
    main loop. Prologue psum comes from the dedicated 2-bank pro pool."""
    negone, neg32, ones_mv = pp["negone"], pp["neg32"], pp["ones_mv"]
    xt_sb, w_sb = handles
    x2col = pp["x2_p"].tile([P, MT], f32)
    handles.append(x2col)
    state = {}
    chunks = []

    def w2_pair(half):
        def emit():
            pro = pp["pro_p"].tile([P, 2, NB], f32, tag="pro", name="pro")
            state[half] = pro
            for i in range(2):
                n = 2 * half + i
                ns = slice(n * NB, (n + 1) * NB)
                for j in range(2):       # k 0..3 via DVE fp8 stt + DR
                    wsq = pp["wsq_p"].tile([P, 2, NB], fp8, tag="wsq8")
                    nc.vector.scalar_tensor_tensor(
                        wsq[:], w_sb[:, 2 * j:2 * j + 2, ns], 32.0,
                        w_sb[:, 2 * j:2 * j + 2, ns], op0=MUL, op1=MUL)
                    nc.tensor.matmul(pro[:, i, :], negone[:], wsq[:],
                                     start=(j == 0), stop=False,
                                     perf_mode=DR, skip_group_check=True)
                for k in range(4, KT):   # k 4..7 via Pool bf16 mul
                    wsq = pp["wsq_p"].tile([P, NB], bf16, tag="wsq16")
                    nc.gpsimd.tensor_mul(wsq[:], w_sb[:, k, ns],
                                         w_sb[:, k, ns])
                    nc.tensor.matmul(pro[:, i, :], neg32[:], wsq[:],
                                     start=False, stop=(k == KT - 1),
                                     skip_group_check=True)
        return emit

    def seeds(half):
        def emit():
            pro = state[half]
            for i in range(2):
                n = 2 * half + i
                ns = slice(n * NB, (n + 1) * NB)
                nc.vector.tensor_scalar_mul(w_sb[0:1, KT - 1, ns],
                                            pro[0:1, i, :], 1.0 / 64.0)
        return emit

    def xsq(h):
        # fp8 squares scaled by 1.75: squares-of-fp8-grid values re-round
        # with a -0.8% systematic bias at scale 1, but near-unbiased at
        # 1.75 (numpy scan); the 1.75 is divided back out in the x2col
        # copy. stt is DVE-only (walrus).
        def emit():
            if h == 0:
                state["xsqs"] = [pp["xsq_p"].tile([P, 2, B], fp8, tag="xsq",
                                                  name=f"xsq{j}")
                                 for j in range(JT)]
            hs = slice(h * (B // 2), (h + 1) * (B // 2))
            for j in range(JT):
                nc.vector.scalar_tensor_tensor(
                    state["xsqs"][j][:, :, hs],
                    xt_sb[:, 2 * j:2 * j + 2, hs], 1.75,
                    xt_sb[:, 2 * j:2 * j + 2, hs], op0=MUL, op1=MUL)
            if h == 1:
                nc.gpsimd.memset(xt_sb[0:1, KT - 1, :], 1.0)  # seed ones row
        return emit

    def x2_groups(h):
        def emit():
            if h == 0:
                state["prox"] = pp["pro_p"].tile([P, 2, NB], f32, tag="pro", name="prox")
            prox = state["prox"]
            for m in range(h * (MT // 2), (h + 1) * (MT // 2)):
                ms = slice(m * P, (m + 1) * P)
                for j in range(JT):
                    nc.tensor.matmul(prox[:, 0, m:m + 1],
                                     state["xsqs"][j][:, :, ms],
                                     ones_mv[:], start=(j == 0),
                                     stop=(j == JT - 1),
                                     perf_mode=DR, skip_group_check=True)
            mh = slice(h * (MT // 2), (h + 1) * (MT // 2))
            nc.vector.tensor_scalar_mul(x2col[:, mh], prox[:, 0, mh],
                                        1.0 / 1.75)
        return emit

    chunks.append(w2_pair(0))
    chunks.append(seeds(0))
    chunks.append(xsq(0))
    chunks.append(w2_pair(1))
    chunks.append(seeds(1))
    chunks.append(xsq(1))
    chunks.append(x2_groups(0))
    chunks.append(x2_groups(1))
    return chunks


PROBE_HALF_K = False     # timing probe: halve the main-matmul work
PROBE_HALF_OUT = False   # timing probe: halve the output DMA bytes


def _emit_main(nc, pp, handles, out_d, interleave):
    """Main loop for one body; `interleave` is the NEXT body's prologue
    chunk list, spread across the m iterations."""
    xt_sb, w_sb, x2col = handles
    jt = JT // 2 if PROBE_HALF_K else JT
    nsteps = len(interleave)
    osb = None
    for m in range(MT):
        if m % 4 == 0:
            osb = pp["o_p"].tile([P, 4, NT, NB], f16)
        psA = pp["ps_p"].tile([P, 2, NB], f32, tag="ps")
        psB = pp["ps_p"].tile([P, 2, NB], f32, tag="ps")
        ms = slice(m * P, (m + 1) * P)
        for j in range(jt):
            for n in range(NT):
                ns = slice(n * NB, (n + 1) * NB)
                ps = psA if n < 2 else psB
                nc.tensor.matmul(ps[:, n % 2, :],
                                 xt_sb[:, 2 * j:2 * j + 2, ms],
                                 w_sb[:, 2 * j:2 * j + 2, ns],
                                 start=(j == 0), stop=(j == jt - 1),
                                 perf_mode=DR, skip_group_check=True)
        nc.scalar.activation(osb[:, m % 4, 0:2], psA[:], AF.Sqrt,
                             bias=x2col[:, m:m + 1], scale=-2.0)
        nc.scalar.activation(osb[:, m % 4, 2:4], psB[:], AF.Sqrt,
                             bias=x2col[:, m:m + 1], scale=-2.0)
        if m % 4 == 3:
            g = m // 4
            if PROBE_HALF_OUT and g % 2 == 1:
                pass
            else:
                # out_d [MT/4, P, 4, OS]: 16KB contiguous per partition
                nc.sync.dma_start(out_d.ap()[g], osb[:])
        # spread the next body's prologue across this body's main loop
        lo = (m * nsteps) // MT
        hi = ((m + 1) * nsteps) // MT
        for c in range(lo, hi):
            interleave[c]()


def build(repeats=1):
    from contextlib import ExitStack
    nc = bacc.Bacc("TRN2", target_bir_lowering=False, debug=False,
                   num_devices=N_CORES)
    xt_d = nc.dram_tensor("xt", [2, P, KT, B // 2], fp8, kind="ExternalInput")
    w_d = nc.dram_tensor("w", [NT, P, KT, NB], fp8, kind="ExternalInput")
    out_d = nc.dram_tensor("out", [MT // 4, P, 4, OS], f16,
                           kind="ExternalOutput")
    with tile.TileContext(nc) as tc:
        with ExitStack() as ctx:
            pp = _make_pools(nc, tc, ctx)
            handles = list(_emit_inputs(nc, tc, pp, xt_d, w_d))
            for c in _prologue_chunks(nc, pp, handles):
                c()
            for r in range(repeats):
                cur = handles
                nxt = []
                if r + 1 < repeats:
                    handles = list(_emit_inputs(nc, tc, pp, xt_d, w_d))
                    nxt = _prologue_chunks(nc, pp, handles)
                _emit_main(nc, pp, cur, out_d, nxt)
    nc.compile()
    return nc


_NC = None


def _fp8_np(a):
    import ml_dtypes
    return np.ascontiguousarray(np.asarray(a).astype(ml_dtypes.float8_e4m3))


def make_in_maps(x, weight):
    # xt [2, P, KT, B/2]: row k*P+p of x.T at [b//(B//2), p, k, b%(B//2)]
    xt8 = _fp8_np(np.asarray(x.T))
    xt8 = np.ascontiguousarray(
        xt8.reshape(KT, P, 2, B // 2).transpose(2, 1, 0, 3))
    maps = []
    for c in range(N_CORES):
        w8 = _fp8_np(weight[:, c * OS:(c + 1) * OS])
        # w [NT, P, KT, NB]: row k*P+p, col n*NB+j at [n, p, k, j]
        w8 = np.ascontiguousarray(
            w8.reshape(KT, P, NT, NB).transpose(2, 1, 0, 3))
        maps.append({"xt": xt8, "w": w8})
    return maps


def _unpack_out(o):
    # out [MT/4, P, 4, OS]: row g*4*P + mm*P + p at [g, p, mm, o]
    return o.transpose(0, 2, 1, 3).reshape(B, OS)


def assemble(results):
    return np.ascontiguousarray(np.concatenate(
        [_unpack_out(results[c]["out"].astype(np.float32))
         for c in range(N_CORES)], axis=1))


def assemble_core0(sim, np_mod):
    o = np_mod.asarray(sim.tensor("out")).astype(np_mod.float32)
    return _unpack_out(o)


def kernel(x, weight):
    global _NC
    x = np.asarray(x, dtype=np.float32)
    weight = np.asarray(weight, dtype=np.float32)
    if _NC is None:
        _NC = build(repeats=1)
    in_maps = make_in_maps(x, weight)
    res = run_bass_kernel_spmd(_NC, in_maps, core_ids=list(range(N_CORES)))
    return assemble(res.results)



# revision 43
# speedup vs baseline: 6.6675x; 1.4193x over previous
"""Euclidean distance layer (retrieval kNN) on 8 Trainium2 NeuronCores.

out[b, o] = || x[b, :] - weight[:, o] ||_2   for x [2048, 1024], weight [1024, 16384].

Strategy (sharding_hint): shard output columns across the 8 cores (2048 each).
Per core, compute d2 = x2[b] + w2[o] - 2 * (x @ w_shard) and out = sqrt(d2):
  - the big matmul runs in fp8e4 with DoubleRow perf mode; each instruction
    contracts a pair of K=128 tiles (its rounding is attenuated ~64x in the
    output because |2xw| << d2); j-outer/n-inner order loads each stationary
    once per (m, k-pair)
  - the -w2/2 seed is FOLDED INTO the last DR pair: the device sets x.T row
    896 (k-tile 7, partition 0) to ones and writes -w2/2 into w row 896
    after computing w2, so every accumulation group gets x2+w2-2xw with no
    extra seed matmul; the dropped x[:,896]*w[896,:] term is ~1e-4 rel
  - w2 = colsum(w^2): DVE squares k 0..3 as fp8 32*w^2 (scalar_tensor_tensor
    is DVE-only — walrus rejects it on Pool) reduced with DoubleRow against
    a -1 stationary; Pool squares k 4..7 as plain bf16 w^2 reduced against a
    -32 bf16 stationary into the same psum group
  - x2 = rowsum(x^2) from fp8 squares of x.T scaled by 1.75 (de-phases the
    fp8 square-of-grid rounding bias; divided back out in the x2col copy),
    used as matmul STATIONARY against a [P,2,1] DR ones moving so each
    k-sum lands straight in a psum column per m-tile
  - epilogue per m-tile: two ACT sqrt(-2*psum + x2_bias) over 2 psum banks
    each ([128,1024]), writing fp16 directly; out DMA every 4 m-tiles
  - no fp16 copy of x is shipped at all: 12MB HBM/body (xt 2 + w 2 + out 8);
    DRAM layouts are host-pre-rearranged so every DMA moves 4-16KB
    contiguous runs per partition (descriptor-efficient)
  - PSUM: main pool 3x2 banks, prologue pool 1x2 banks. The prologue pool
    frees early in each body, so body i+1's whole w2/x2/seed prologue is
    emitted INTERLEAVED into body i's main loop and runs during it —
    without this the ACT stream starves ~10us at every body boundary
  - input/output/x2col pools are triple-buffered so body i+2's DMAs carry
    no tile WAR wait and fill every idle slot of the DMA device; out DMAs
    issue from GPSIMD (SWDGE) so SP's in-order queue carries only inputs
Host side only transposes/shards/casts inputs and reassembles the output.
"""
import numpy as np

import concourse.bass as bass
import concourse.tile as tile
from concourse import bacc, mybir
from concourse.bass_utils import run_bass_kernel_spmd

f32 = mybir.dt.float32
f16 = mybir.dt.float16
bf16 = mybir.dt.bfloat16
fp8 = mybir.dt.float8e4
AF = mybir.ActivationFunctionType
MUL = mybir.AluOpType.mult

B = 2048      # batch rows
I = 1024      # input size (contraction)
O = 16384     # output size (prototype count)
N_CORES = 8
OS = O // N_CORES   # 2048 output columns per core
P = 128       # partitions
NB = 512      # psum bank width in f32
KT = I // P   # 8 k-tiles
MT = B // P   # 16 m-tiles
NT = OS // NB  # 4 n-blocks
JT = KT // 2  # 4 DoubleRow k-pairs

DR = mybir.MatmulPerfMode.DoubleRow


def _make_pools(nc, tc, ctx):
    pools = dict(
        const_p=ctx.enter_context(tc.tile_pool(name="const", bufs=1)),
        xt_p=ctx.enter_context(tc.tile_pool(name="xt", bufs=3)),
        w_p=ctx.enter_context(tc.tile_pool(name="w", bufs=3)),
        wsq_p=ctx.enter_context(tc.tile_pool(name="wsq", bufs=5)),
        xsq_p=ctx.enter_context(tc.tile_pool(name="xsq", bufs=5)),
        x2_p=ctx.enter_context(tc.tile_pool(name="x2", bufs=3)),
        o_p=ctx.enter_context(tc.tile_pool(name="o", bufs=3)),
        ps_p=ctx.enter_context(tc.tile_pool(name="ps", bufs=3, space="PSUM")),
        pro_p=ctx.enter_context(tc.tile_pool(name="pro", bufs=1, space="PSUM")),
    )
    negone = pools["const_p"].tile([P, 2, P], fp8)  # w2 sum, DR over 32*w^2
    nc.vector.memset(negone[:], -1.0)
    neg32 = pools["const_p"].tile([P, P], bf16)     # w2 sum, bf16 over w^2
    nc.vector.memset(neg32[:], -32.0)
    ones_mv = pools["const_p"].tile([P, 2, 1], fp8)  # DR moving for x2 sum
    nc.vector.memset(ones_mv[:], 1.0)
    pools["negone"] = negone
    pools["neg32"] = neg32
    pools["ones_mv"] = ones_mv
    return pools


def _emit_inputs(nc, tc, pp, xt_d, w_d):
    """Allocate this body's input tiles and issue their DMAs at high
    scheduler priority so the next body's inputs transfer during the
    current body's DMA-idle window."""
    xt_sb = pp["xt_p"].tile([P, KT, B], fp8)    # x.T resident
    w_sb = pp["w_p"].tile([P, KT, OS], fp8)     # w shard resident

    # DRAM layouts are host-pre-rearranged so every DMA reads 4-8KB
    # contiguous runs per partition (descriptor-efficient):
    #   xt_d [2, P, KT, B/2] (b-half major), w_d [NT, P, KT, NB]
    with tc.high_priority(offset=800):
        nc.sync.dma_start(xt_sb[:, :, 0:B // 2], xt_d.ap()[0])
        for n in range(NT):
            ns = slice(n * NB, (n + 1) * NB)
            nc.sync.dma_start(w_sb[:, :, ns], w_d.ap()[n])
            if n == 0:
                nc.sync.dma_start(xt_sb[:, :, B // 2:B], xt_d.ap()[1])
    return xt_sb, w_sb


def _prologue_chunks(nc, pp, handles):
    """Build the w2/seed/x2 prologue for one body as a list of emission
    closures, so the caller can interleave them into the previous body's
    main loop. Prologue psum comes from the dedicated 2-bank pro pool."""
    negone, neg32, ones_mv = pp["negone"], pp["neg32"], pp["ones_mv"]
    xt_sb, w_sb = handles
    x2col = pp["x2_p"].tile([P, MT], f32)
    handles.append(x2col)
    state = {}
    chunks = []

    def w2_pair(half):
        def emit():
            pro = pp["pro_p"].tile([P, 2, NB], f32, tag="pro", name="pro")
            state[half] = pro
            for i in range(2):
                n = 2 * half + i
                ns = slice(n * NB, (n + 1) * NB)
                for j in range(2):       # k 0..3 via DVE fp8 stt + DR
                    wsq = pp["wsq_p"].tile([P, 2, NB], fp8, tag="wsq8")
                    nc.vector.scalar_tensor_tensor(
                        wsq[:], w_sb[:, 2 * j:2 * j + 2, ns], 32.0,
                        w_sb[:, 2 * j:2 * j + 2, ns], op0=MUL, op1=MUL)
                    nc.tensor.matmul(pro[:, i, :], negone[:], wsq[:],
                                     start=(j == 0), stop=False,
                                     perf_mode=DR, skip_group_check=True)
                for k in range(4, KT):   # k 4..7 via Pool bf16 mul
                    wsq = pp["wsq_p"].tile([P, NB], bf16, tag="wsq16")
                    nc.gpsimd.tensor_mul(wsq[:], w_sb[:, k, ns],
                                         w_sb[:, k, ns])
                    nc.tensor.matmul(pro[:, i, :], neg32[:], wsq[:],
                                     start=False, stop=(k == KT - 1),
                                     skip_group_check=True)
        return emit

    def seeds(half):
        def emit():
            pro = state[half]
            for i in range(2):
                n = 2 * half + i
                ns = slice(n * NB, (n + 1) * NB)
                nc.vector.tensor_scalar_mul(w_sb[0:1, KT - 1, ns],
                                            pro[0:1, i, :], 1.0 / 64.0)
        return emit

    def xsq(h):
        # fp8 squares scaled by 1.75: squares-of-fp8-grid values re-round
        # with a -0.8% systematic bias at scale 1, but near-unbiased at
        # 1.75 (numpy scan); the 1.75 is divided back out in the x2col
        # copy. stt is DVE-only (walrus).
        def emit():
            if h == 0:
                state["xsqs"] = [pp["xsq_p"].tile([P, 2, B], fp8, tag="xsq",
                                                  name=f"xsq{j}")
                                 for j in range(JT)]
            hs = slice(h * (B // 2), (h + 1) * (B // 2))
            for j in range(JT):
                nc.vector.scalar_tensor_tensor(
                    state["xsqs"][j][:, :, hs],
                    xt_sb[:, 2 * j:2 * j + 2, hs], 1.75,
                    xt_sb[:, 2 * j:2 * j + 2, hs], op0=MUL, op1=MUL)
            if h == 1:
                nc.gpsimd.memset(xt_sb[0:1, KT - 1, :], 1.0)  # seed ones row
        return emit

    def x2_groups(h):
        def emit():
            if h == 0:
                state["prox"] = pp["pro_p"].tile([P, 2, NB], f32, tag="pro", name="prox")
            prox = state["prox"]
            for m in range(h * (MT // 2), (h + 1) * (MT // 2)):
                ms = slice(m * P, (m + 1) * P)
                for j in range(JT):
                    nc.tensor.matmul(prox[:, 0, m:m + 1],
                                     state["xsqs"][j][:, :, ms],
                                     ones_mv[:], start=(j == 0),
                                     stop=(j == JT - 1),
                                     perf_mode=DR, skip_group_check=True)
            mh = slice(h * (MT // 2), (h + 1) * (MT // 2))
            nc.vector.tensor_scalar_mul(x2col[:, mh], prox[:, 0, mh],
                                        1.0 / 1.75)
        return emit

    chunks.append(w2_pair(0))
    chunks.append(seeds(0))
    chunks.append(xsq(0))
    chunks.append(w2_pair(1))
    chunks.append(seeds(1))
    chunks.append(xsq(1))
    chunks.append(x2_groups(0))
    chunks.append(x2_groups(1))
    return chunks


PROBE_HALF_K = False     # timing probe: halve the main-matmul work
PROBE_HALF_OUT = False   # timing probe: halve the output DMA bytes


def _emit_main(nc, pp, handles, out_d, interleave):
    """Main loop for one body; `interleave` is the NEXT body's prologue
    chunk list, spread across the m iterations."""
    xt_sb, w_sb, x2col = handles
    jt = JT // 2 if PROBE_HALF_K else JT
    nsteps = len(interleave)
    osb = None
    for m in range(MT):
        if m % 4 == 0:
            osb = pp["o_p"].tile([P, 4, NT, NB], f16)
        psA = pp["ps_p"].tile([P, 2, NB], f32, tag="ps")
        psB = pp["ps_p"].tile([P, 2, NB], f32, tag="ps")
        ms = slice(m * P, (m + 1) * P)
        for j in range(jt):
            for n in range(NT):
                ns = slice(n * NB, (n + 1) * NB)
                ps = psA if n < 2 else psB
                nc.tensor.matmul(ps[:, n % 2, :],
                                 xt_sb[:, 2 * j:2 * j + 2, ms],
                                 w_sb[:, 2 * j:2 * j + 2, ns],
                                 start=(j == 0), stop=(j == jt - 1),
                                 perf_mode=DR, skip_group_check=True)
        nc.scalar.activation(osb[:, m % 4, 0:2], psA[:], AF.Sqrt,
                             bias=x2col[:, m:m + 1], scale=-2.0)
        nc.scalar.activation(osb[:, m % 4, 2:4], psB[:], AF.Sqrt,
                             bias=x2col[:, m:m + 1], scale=-2.0)
        if m % 4 == 3:
            g = m // 4
            if PROBE_HALF_OUT and g % 2 == 1:
                pass
            else:
                # out_d [MT/4, P, 4, OS]: 16KB contiguous per partition
                nc.gpsimd.dma_start(out_d.ap()[g], osb[:])
        # spread the next body's prologue across this body's main loop
        lo = (m * nsteps) // MT
        hi = ((m + 1) * nsteps) // MT
        for c in range(lo, hi):
            interleave[c]()


def build(repeats=1):
    from contextlib import ExitStack
    nc = bacc.Bacc("TRN2", target_bir_lowering=False, debug=False,
                   num_devices=N_CORES)
    xt_d = nc.dram_tensor("xt", [2, P, KT, B // 2], fp8, kind="ExternalInput")
    w_d = nc.dram_tensor("w", [NT, P, KT, NB], fp8, kind="ExternalInput")
    out_d = nc.dram_tensor("out", [MT // 4, P, 4, OS], f16,
                           kind="ExternalOutput")
    with tile.TileContext(nc) as tc:
        with ExitStack() as ctx:
            pp = _make_pools(nc, tc, ctx)
            handles = list(_emit_inputs(nc, tc, pp, xt_d, w_d))
            for c in _prologue_chunks(nc, pp, handles):
                c()
            for r in range(repeats):
                cur = handles
                nxt = []
                if r + 1 < repeats:
                    handles = list(_emit_inputs(nc, tc, pp, xt_d, w_d))
                    nxt = _prologue_chunks(nc, pp, handles)
                _emit_main(nc, pp, cur, out_d, nxt)
    nc.compile()
    return nc


_NC = None


def _fp8_np(a):
    import ml_dtypes
    return np.ascontiguousarray(np.asarray(a).astype(ml_dtypes.float8_e4m3))


def make_in_maps(x, weight):
    # xt [2, P, KT, B/2]: row k*P+p of x.T at [b//(B//2), p, k, b%(B//2)]
    xt8 = _fp8_np(np.asarray(x.T))
    xt8 = np.ascontiguousarray(
        xt8.reshape(KT, P, 2, B // 2).transpose(2, 1, 0, 3))
    maps = []
    for c in range(N_CORES):
        w8 = _fp8_np(weight[:, c * OS:(c + 1) * OS])
        # w [NT, P, KT, NB]: row k*P+p, col n*NB+j at [n, p, k, j]
        w8 = np.ascontiguousarray(
            w8.reshape(KT, P, NT, NB).transpose(2, 1, 0, 3))
        maps.append({"xt": xt8, "w": w8})
    return maps


def _unpack_out(o):
    # out [MT/4, P, 4, OS]: row g*4*P + mm*P + p at [g, p, mm, o]
    return o.transpose(0, 2, 1, 3).reshape(B, OS)


def assemble(results):
    return np.ascontiguousarray(np.concatenate(
        [_unpack_out(results[c]["out"].astype(np.float32))
         for c in range(N_CORES)], axis=1))


def assemble_core0(sim, np_mod):
    o = np_mod.asarray(sim.tensor("out")).astype(np_mod.float32)
    return _unpack_out(o)


def kernel(x, weight):
    global _NC
    x = np.asarray(x, dtype=np.float32)
    weight = np.asarray(weight, dtype=np.float32)
    if _NC is None:
        _NC = build(repeats=1)
    in_maps = make_in_maps(x, weight)
    res = run_bass_kernel_spmd(_NC, in_maps, core_ids=list(range(N_CORES)))
    return assemble(res.results)

